# revision 31
# baseline (speedup 1.0000x reference)
import os
import sys

os.environ.setdefault("JAX_PLATFORMS", "")
sys.path.insert(0, "/opt/trn_rl_repo")

import numpy as np
import ml_dtypes

BF16 = ml_dtypes.bfloat16
INV = np.float32(1.0 / np.sqrt(1.0 + 1e-5))
G8 = [[0, 1, 2, 3, 4, 5, 6, 7]]
G4 = [[0, 1, 2, 3], [4, 5, 6, 7]]
AG4 = True
FP8 = True
NOCOLL = os.environ.get("V2_NOCOLL", "0") == "1"
NLOC = 1024
CHUNKS = (0, 512)

_CACHE = {}


def _blockP(a):
    a = np.ascontiguousarray(a)
    R, C = a.shape
    assert R % 128 == 0, (R, C)
    nb = R // 128
    return np.ascontiguousarray(a.reshape(nb, 128, C).transpose(1, 0, 2).reshape(128, nb * C))


# name -> (shape, "bf"/"f32"/"u32")
SPECS = {
    "xT": ((3, 1024), "bf"),
    "c1T": ((3, 64), "bf"),
    "c2T": ((64, 128), "bf"),
    "c3T": ((128, 256), "bf"),
    "p1T": ((128, 512), "bf"),
    "p2T": ((128, 512), "bf"),
    "fb1": ((128, 1), "f32"), "fb2": ((128, 1), "f32"), "fb3": ((128, 2), "f32"),
    "fbp1": ((128, 2), "f32"), "fbp2": ((128, 2), "f32"),
    "qkT0": ((128, 256), "bf"), "qkT1": ((128, 256), "bf"),
    "vwT0": ((128, 512), "bf"), "vwT1": ((128, 512), "bf"),
    "twT0": ((128, 512), "bf"), "twT1": ((128, 512), "bf"),
    "vbb0": ((128, 256), "f32"), "vbb1": ((128, 256), "f32"),
    "tbf0": ((128, 2), "f32"), "tbf1": ((128, 2), "f32"),
    "cfT": ((128, 4096), "bf"), "cfb": ((128, 4), "f32"),
    "s1fT": ((128, 2048), "bf"), "s1gT": ((128, 2048), "bf"), "s1b": ((128, 4), "f32"),
    "s2T": ((128, 1024), "bf"), "s2b": ((128, 2), "f32"),
    "s3T": ((128, 100), "bf"),
}

# pack every weight (everything but per-core xT) into two mega input
# tensors so each call ships 3 PJRT buffers instead of ~34
BF_OFFS, F32_OFFS = {}, {}
_bo = _fo = 0
for _nm, (_sh, _dn) in SPECS.items():
    if _nm == "xT":
        continue
    if _dn == "bf":
        BF_OFFS[_nm] = _bo
        _bo += _sh[1]
    else:
        F32_OFFS[_nm] = _fo
        _fo += _sh[1]
BF_TOT, F32_TOT = _bo, _fo


DBG_SPECS = {
    "d_front": ((128, 2048), "f32"),
    "d_xq0": ((128, 1024), "bf"),
    "d_xqf": ((128, 4096), "bf"),
    "d_xvtf": ((128, 8192), "bf"),
    "d_rsg": ((128, 32), "f32"),
    "d_cs": ((1, 1024), "f32"),
    "d_xrh": ((128, 2048), "bf"),
    "d_xm1": ((128, 2048), "f32"),
    "d_xm4": ((128, 2048), "f32"),
    "d_gmg": ((128, 4), "f32"),
    "d_h2h": ((128, 4096), "bf"),
}


def _build(debug=False, reps=1):
    from concourse import tile, bacc, mybir
    from concourse.bass import ds

    dt = mybir.dt
    AF = mybir.ActivationFunctionType
    AX = mybir.AxisListType
    ALU = mybir.AluOpType
    bf, f32 = dt.bfloat16, dt.float32

    nc = bacc.Bacc("TRN2", target_bir_lowering=False, debug=False, num_devices=8)

    WB = nc.dram_tensor("WB", [128, BF_TOT], bf, kind="ExternalInput")
    WF = nc.dram_tensor("WF", [128, F32_TOT], f32, kind="ExternalInput")
    XT = nc.dram_tensor("xT", [3, 1024], bf, kind="ExternalInput")
    out_d = nc.dram_tensor("out", [50, 1024], f32, kind="ExternalOutput")
    DBG = {}
    if debug:
        for nm, (sh, dn) in DBG_SPECS.items():
            DBG[nm] = nc.dram_tensor(nm, list(sh), bf if dn == "bf" else f32,
                                     kind="ExternalOutput")

    def tap(nm, t):
        if debug and nm in DBG:
            nc.gpsimd.dma_start(DBG[nm][:], t[:])

    with tile.TileContext(nc) as tc:
        with (
            tc.tile_pool(name="pers", bufs=1) as pers,
            tc.tile_pool(name="dramp", bufs=1, space="DRAM") as dramp,
        ):
            def sload(pool, nm, eng=None):
                sh, dn = SPECS[nm]
                dty = {"bf": bf, "f32": f32}[dn]
                t = pool.tile(list(sh), dty, tag=nm, name=nm + "_sb")
                if nm == "xT":
                    src = XT[:]
                elif dn == "bf":
                    src = WB[:][0: sh[0], BF_OFFS[nm]: BF_OFFS[nm] + sh[1]]
                else:
                    src = WF[:][0: sh[0], F32_OFFS[nm]: F32_OFFS[nm] + sh[1]]
                (eng or nc.sync).dma_start(t[:], src)
                return t

            S = {}
            # SA weights on gpsimd queue, back weights on scalar queue
            # (front weights go on sync in fp below)
            for nm in ("qkT0", "qkT1", "vwT0", "vwT1", "twT0", "twT1",
                       "vbb0", "vbb1", "tbf0", "tbf1"):
                S[nm] = sload(pers, nm, eng=nc.gpsimd)
            for nm in ("cfT", "cfb", "s1fT", "s1gT", "s1b", "s2T", "s2b", "s3T"):
                S[nm] = sload(pers, nm, eng=nc.scalar)


            xm = pers.tile([128, 2048], f32, tag="xm")
            slot_h = [pers.tile([128, 2048], bf, tag=f"sh{s}", name=f"sh{s}") for s in range(5)]
            ones1 = pers.tile([1, 128], f32, tag="ones1")
            nc.gpsimd.memset(ones1[:], 1.0)

            agq_in = dramp.tile([128, 1024], dt.float8e4 if FP8 else bf, tag="agq_in")
            agq_out = dramp.tile([512 if AG4 else 1024, 1024], dt.float8e4 if FP8 else bf, tag="agq_out")
            agv_in = dramp.tile([128, 2048], dt.float8e4 if FP8 else bf, tag="agv_in")
            agv_out = dramp.tile([512 if AG4 else 1024, 2048], dt.float8e4 if FP8 else bf, tag="agv_out")
            ar_in = dramp.tile([128, 16], f32, tag="ar_in")
            ar_out = dramp.tile([128, 16], f32, tag="ar_out")
            ar_in2 = dramp.tile([128, 16], f32, tag="ar_in2")
            ar_out2 = dramp.tile([128, 16], f32, tag="ar_out2")
            gm_in = dramp.tile([128, 4], f32, tag="gm_in")
            gm_out = dramp.tile([128, 4], f32, tag="gm_out")

            import itertools
            _regctr = itertools.count()

            def _pair_ap(t, gp, c0):
                return t[:].rearrange("p (g n) -> p g n", n=1024)[:, 2 * gp: 2 * gp + 2, c0: c0 + 512]

            def _pair_lhs(t, gp, cb):
                return t[:].rearrange("p (g c) -> p g c", c=256)[:, 2 * gp: 2 * gp + 2, cb * 128: cb * 128 + 128]

            def _pair_rb(t, gp):
                return t[:].rearrange("p (g o) -> p g o", o=16)[:, 2 * gp: 2 * gp + 2, 0:1]

            def emit_pipeline():
                # ---------------- front chain ----------------
                with (
                    tc.tile_pool(name="fp", bufs=1) as fp,
                    tc.tile_pool(name="fps", bufs=1, space="PSUM") as fps,
                ):
                    for nm in ("xT", "c1T", "c2T", "c3T", "p1T", "p2T",
                               "fb1", "fb2", "fb3", "fbp1", "fbp2"):
                        S[nm] = sload(fp, nm)

                    chain = [
                        ("c1T", 1, 64, "fb1"),
                        ("c2T", 1, 128, "fb2"),
                        ("c3T", 1, 256, "fb3"),
                        ("p1T", 2, 256, "fbp1"),
                        ("p2T", 2, 256, "fbp2"),
                    ]
                    cur = S["xT"]
                    for li, (wn, kb, Cout, bn) in enumerate(chain):
                        w, bt = S[wn], S[bn]
                        nob = (Cout + 127) // 128
                        P_out = min(Cout, 128)
                        last = li == len(chain) - 1
                        if last:
                            of = xm
                        else:
                            of = fp.tile([P_out, nob * 1024], f32, tag=f"hf{li}", name=f"hf{li}")
                        for ob in range(nob):
                            Mob = min(128, Cout - 128 * ob)
                            ps = fps.tile([Mob, 1024], f32, tag="fps_t", bufs=2, name="fps_t")
                            for c0 in CHUNKS:
                                for kbi in range(kb):
                                    nc.tensor.matmul(
                                        ps[:, c0:c0 + 512],
                                        w[:, kbi * Cout + 128 * ob: kbi * Cout + 128 * ob + Mob],
                                        cur[:, kbi * 1024 + c0: kbi * 1024 + c0 + 512],
                                        start=(kbi == 0), stop=(kbi == kb - 1))
                            nc.scalar.activation(
                                of[0:Mob, ob * 1024: ob * 1024 + 1024], ps[:],
                                AF.Relu, bias=bt[0:Mob, ob: ob + 1],
                            )
                        if last:
                            nh = slot_h[0]
                        else:
                            nh = fp.tile([P_out, nob * 1024], bf, tag=f"hh{li}", name=f"hh{li}")
                        W = nob * 1024
                        nc.vector.tensor_copy(nh[0:P_out, 0:W], of[0:P_out, 0:W])
                        cur = nh
                tap("d_front", xm)

                # ---------------- 4 SA layers ----------------
                with tc.tile_pool(name="sap", bufs=1) as sp:
                    for L in range(4):
                        v = 0 if L == 0 else 1
                        ih = slot_h[L]
                        oh = slot_h[L + 1]
                        qh = S[f"qkT{v}"]
                        vh = S[f"vwT{v}"]
                        th = S[f"twT{v}"]
                        vbb, tbt = S[f"vbb{v}"], S[f"tbf{v}"]

                        xq_loc = sp.tile([128, 1024], dt.float8e4 if FP8 else bf, tag="xq_loc", name="xq_loc")
                        xvt_loc = sp.tile([128, 2048], dt.float8e4 if FP8 else bf, tag="xvt_loc", name="xvt_loc")
                        xq_full = sp.tile([128, 4096], dt.float8e4 if FP8 else bf, tag="xq_full", name="xq_full")
                        xvt_full = sp.tile([128, 8192], dt.float8e4 if FP8 else bf, tag="xvt_full", name="xvt_full")
                        P_sb = sp.tile([128, 32 * 1024], dt.float8e4 if FP8 else bf, tag="P_sb", name="P_sb")
                        rs_sb = sp.tile([128, 32], f32, tag="rs_sb", name="rs_sb")
                        rs_g = sp.tile([128, 32], f32, tag="rs_g", name="rs_g")
                        r_f = sp.tile([128, 32], f32, tag="r_f", name="r_f")
                        r_b = sp.tile([128, 32], bf, tag="r_b", name="r_b")
                        xr_hi = sp.tile([128, 2048], bf, tag="xr_hi", name="xr_hi")
                        dxr = sp.tile([128, 2048], bf, tag="dxr", name="dxr")
                        if FP8:
                            xvt_f8 = sp.tile([128, 8192], dt.float8e4, tag="xvt_f8", name="xvt_f8")
                            r_b8 = sp.tile([128, 512], dt.float8e4, tag="r_b8", name="r_b8")
                        icb = sp.tile([128, 1024], f32, tag="icb", name="icb")
                        cs_row = sp.tile([1, 1024], f32, tag="cs_row", name="cs_row")
                        ic_row = sp.tile([1, 1024], f32, tag="ic_row", name="ic_row")

                        if not AG4:
                            greg = nc.sync.alloc_register(f"greg{L}_{next(_regctr)}")
                            nc.sync.reg_load(greg, S["gbase"][0:1, 0:1])
                            gbase = nc.sync.snap(greg, donate=True, min_val=0, max_val=512)

                        # phase 0: xq first, ship AG1 asap
                        with tc.tile_pool(name="ps0", bufs=1, space="PSUM") as ps0:
                            qps = ps0.tile([128, 1024], f32, tag="qps", name="qps")
                            for c0 in CHUNKS:
                                for kbi in range(2):
                                    nc.tensor.matmul(
                                        qps[:, c0: c0 + 512], qh[:, kbi * 128: kbi * 128 + 128],
                                        ih[:, kbi * 1024 + c0: kbi * 1024 + c0 + 512],
                                        start=(kbi == 0), stop=(kbi == 1))
                            nc.scalar.activation(xq_loc[:], qps[:], AF.Copy)
                            nc.sync.dma_start(agq_in[:], xq_loc[:])
                            if NOCOLL:
                                for k in range(4):
                                    nc.gpsimd.dma_start(agq_out[k * 128:(k + 1) * 128, :], agq_in[:])
                            else:
                                nc.gpsimd.collective_compute(
                                    "AllGather", ALU.bypass, replica_groups=G4 if AG4 else G8,
                                    ins=[agq_in.opt()], outs=[agq_out.opt()],
                                )
                            # xvt, then AG2
                            for nb in range(8):
                                vps = ps0.tile([128, 256], f32, tag="vps", bufs=2, name="vps")
                                for kbi in range(2):
                                    nc.tensor.matmul(
                                        vps[:], ih[:, kbi * 1024 + nb * 128: kbi * 1024 + nb * 128 + 128],
                                        vh[:, kbi * 256: kbi * 256 + 256],
                                        start=(kbi == 0), stop=(kbi == 1))
                                nc.vector.tensor_add(xvt_loc[:, nb * 256: nb * 256 + 256], vps[:], vbb[:])
                            nc.sync.dma_start(agv_in[:], xvt_loc[:])
                            if NOCOLL:
                                for k in range(4):
                                    nc.gpsimd.dma_start(agv_out[k * 128:(k + 1) * 128, :], agv_in[:])
                            else:
                                nc.gpsimd.collective_compute(
                                    "AllGather", ALU.bypass, replica_groups=G4 if AG4 else G8,
                                    ins=[agv_in.opt()], outs=[agv_out.opt()],
                                )


                        # unstage xq (this layer's AG1) then xvt (AG2)
                        if AG4:
                            nc.sync.dma_start(xq_full[:].rearrange("p (k j) -> p k j", k=4),
                                              agq_out[:].rearrange("(k p) j -> p k j", p=128))
                            nc.sync.dma_start(xvt_full[:].rearrange("p (k j) -> p k j", k=4),
                                              agv_out[:].rearrange("(k p) j -> p k j", p=128))
                        else:
                            for k in range(4):
                                nc.sync.dma_start(xq_full[:, k * 1024: (k + 1) * 1024],
                                                  agq_out[ds(gbase + k * 128, 128), :])
                            for k in range(4):
                                nc.sync.dma_start(xvt_full[:, k * 2048: (k + 1) * 2048],
                                                  agv_out[ds(gbase + k * 128, 128), :])

                        if L == 0:
                            tap("d_xq0", xq_loc)

                        # phase 2: energy + exp(P); rowsums via DVE reduce from P_sb
                        with tc.tile_pool(name="ps2", bufs=1, space="PSUM") as ps2:
                            for gp in range(16):
                                eps = ps2.tile([128, 2048], f32, tag="eps", bufs=2, name="eps")
                                for gi in range(2):
                                    g = 2 * gp + gi
                                    lhs = xq_full[:, g * 128: (g + 1) * 128]
                                    for c0 in CHUNKS:
                                        nc.tensor.matmul(eps[:, gi * 1024 + c0: gi * 1024 + c0 + 512],
                                                         lhs, xq_loc[:, c0: c0 + 512],
                                                         start=True, stop=True)
                                for gi in range(2):
                                    g = 2 * gp + gi
                                    nc.scalar.activation(P_sb[:, g * 1024: g * 1024 + 1024],
                                                         eps[:, gi * 1024: gi * 1024 + 1024],
                                                         AF.Exp, accum_out=rs_sb[:, g: g + 1])
                                if gp == 7:
                                    nc.sync.dma_start(ar_in[:], rs_sb[:, 0:16])
                                    if NOCOLL:
                                        nc.gpsimd.dma_start(ar_out[:], ar_in[:])
                                    else:
                                        nc.gpsimd.collective_compute(
                                            "AllReduce", ALU.add, replica_groups=G4,
                                            ins=[ar_in.opt()], outs=[ar_out.opt()],
                                        )

                        if L == 0:
                            tap("d_xqf", xq_full)

                        # phase 3: second-half rowsum allreduce (first half ran under exp)
                        nc.sync.dma_start(ar_in2[:], rs_sb[:, 16:32])
                        if NOCOLL:
                            nc.gpsimd.dma_start(ar_out2[:], ar_in2[:])
                        else:
                            nc.gpsimd.collective_compute(
                                "AllReduce", ALU.add, replica_groups=G4,
                                ins=[ar_in2.opt()], outs=[ar_out2.opt()],
                            )
                        nc.sync.dma_start(rs_g[:, 0:16], ar_out[:])
                        nc.sync.dma_start(rs_g[:, 16:32], ar_out2[:])
                        if FP8:
                            nc.scalar.mul(rs_g[:], rs_g[:], 1.0 / 1024.0)
                        nc.vector.reciprocal(r_f[:], rs_g[:])
                        if FP8:
                            nc.vector.tensor_copy(
                                r_b8[:].rearrange("p (g o) -> p g o", o=16)[:, :, 0:1],
                                r_f[:].unsqueeze(2))
                        else:
                            nc.vector.tensor_copy(r_b[:], r_f[:])

                        with tc.tile_pool(name="ps3", bufs=1, space="PSUM") as ps3:
                            # colsum (needs r_b + P only) runs while vector scales xvt
                            for ci, c0 in enumerate(CHUNKS):
                                csp = ps3.tile([1, 512], f32, tag=f"csp{ci}", name=f"csp{ci}")
                                if FP8:
                                    for gp in range(16):
                                        nc.tensor.matmul(
                                            csp[:], _pair_rb(r_b8, gp),
                                            _pair_ap(P_sb, gp, c0),
                                            start=(gp == 0), stop=(gp == 15),
                                            perf_mode=mybir.MatmulPerfMode.DoubleRow)
                                else:
                                    for g in range(32):
                                        nc.tensor.matmul(csp[:], r_b[:, g: g + 1],
                                                         P_sb[:, g * 1024 + c0: g * 1024 + c0 + 512],
                                                         start=(g == 0), stop=(g == 31))
                                nc.vector.tensor_scalar_add(cs_row[:, c0: c0 + 512], csp[:], 1e-9)
                            nc.vector.reciprocal(ic_row[:], cs_row[:])
                            for g in range(32):
                                if FP8:
                                    nc.vector.tensor_scalar_mul(
                                        xvt_f8[:, g * 256: (g + 1) * 256],
                                        xvt_full[:, g * 256: (g + 1) * 256], r_f[:, g: g + 1])
                                else:
                                    nc.vector.tensor_scalar_mul(
                                        xvt_full[:, g * 256: (g + 1) * 256],
                                        xvt_full[:, g * 256: (g + 1) * 256], r_f[:, g: g + 1])
                            for c0 in CHUNKS:
                                ibp = ps3.tile([128, 512], f32, tag="ibp", bufs=2, name="ibp")
                                nc.tensor.matmul(ibp[:], ones1[:], ic_row[:, c0: c0 + 512], start=True, stop=True)
                                nc.scalar.activation(icb[:, c0: c0 + 512], ibp[:], AF.Copy)
                            for cb in range(2):
                                for c0 in CHUNKS:
                                    xrp = ps3.tile([128, 512], f32, tag="xrp", bufs=2, name="xrp")
                                    if FP8:
                                        for gp in range(16):
                                            nc.tensor.matmul(
                                                xrp[:],
                                                _pair_lhs(xvt_f8, gp, cb),
                                                _pair_ap(P_sb, gp, c0),
                                                start=(gp == 0), stop=(gp == 15),
                                                perf_mode=mybir.MatmulPerfMode.DoubleRow)
                                    else:
                                        for g in range(32):
                                            nc.tensor.matmul(
                                                xrp[:],
                                                xvt_full[:, g * 256 + cb * 128: g * 256 + cb * 128 + 128],
                                                P_sb[:, g * 1024 + c0: g * 1024 + c0 + 512],
                                                start=(g == 0), stop=(g == 31))
                                    sl = slice(cb * 1024 + c0, cb * 1024 + c0 + 512)
                                    nc.scalar.activation(xr_hi[:, sl], xrp[:], AF.Copy)
                            # xr *= icb (per local column), then d = x - xr
                            for cb in range(2):
                                csl = slice(cb * 1024, cb * 1024 + 1024)
                                nc.vector.tensor_mul(xr_hi[:, csl], xr_hi[:, csl], icb[:])
                            nc.vector.tensor_sub(dxr[:], ih[:], xr_hi[:])

                        if L == 0:
                            tap("d_rsg", rs_g)
                            tap("d_cs", cs_row)
                            tap("d_xrh", xr_hi)
                            tap("d_xvtf", xvt_full)

                        # phase 4: y = relu(tw@(x - xr) + tbf), resid add
                        with tc.tile_pool(name="ps4", bufs=1, space="PSUM") as ps4:
                            for ob in range(2):
                                for c0 in CHUNKS:
                                    bps = ps4.tile([128, 512], f32, tag="bps", bufs=2, name="bps")
                                    for kbi in range(2):
                                        nc.tensor.matmul(
                                            bps[:], th[:, kbi * 256 + 128 * ob: kbi * 256 + 128 * ob + 128],
                                            dxr[:, kbi * 1024 + c0: kbi * 1024 + c0 + 512],
                                            start=(kbi == 0), stop=(kbi == 1))
                                    osl = slice(ob * 1024 + c0, ob * 1024 + c0 + 512)
                                    yv = sp.tile([128, 512], f32, tag="scr", bufs=2, name="yv")
                                    nc.scalar.activation(yv[:], bps[:], AF.Relu, bias=tbt[:, ob: ob + 1])
                                    if L == 3:
                                        nc.vector.tensor_add(oh[:, osl], xm[:, osl], yv[:])
                                    else:
                                        nc.vector.tensor_add(xm[:, osl], xm[:, osl], yv[:])
                                        nc.vector.tensor_copy(oh[:, osl], xm[:, osl])
                        if L == 0:
                            tap("d_xm1", xm)
                        if L == 3:
                            tap("d_xm4", xm)

                # ---------------- back end ----------------
                with tc.tile_pool(name="bp", bufs=1) as bp:
                    face_hi = bp.tile([128, 4096], bf, tag="face_hi", name="face_hi")
                    gml = bp.tile([128, 4], f32, tag="gml", name="gml")

                    with tc.tile_pool(name="psA", bufs=1, space="PSUM") as psA:
                        for ob in range(4):
                            for c0 in CHUNKS:
                                fpt = psA.tile([128, 512], f32, tag="fpsb", bufs=2, name="fpt")
                                for sk in range(8):
                                    s, cb = 1 + sk // 2, sk % 2
                                    nc.tensor.matmul(
                                        fpt[:],
                                        S["cfT"][:, sk * 512 + 128 * ob: sk * 512 + 128 * ob + 128],
                                        slot_h[s][:, cb * 1024 + c0: cb * 1024 + c0 + 512],
                                        start=(sk == 0), stop=(sk == 7))
                                nc.scalar.activation(face_hi[:, ob * 1024 + c0: ob * 1024 + c0 + 512],
                                                     fpt[:], AF.Prelu,
                                                     bias=S["cfb"][:, ob: ob + 1], alpha=0.2)
                            nc.vector.tensor_reduce(gml[:, ob: ob + 1],
                                                    face_hi[:, ob * 1024: (ob + 1) * 1024],
                                                    axis=AX.X, op=ALU.max)

                        nc.sync.dma_start(gm_in[:], gml[:])
                        if NOCOLL:
                            nc.gpsimd.dma_start(gm_out[:], gm_in[:])
                        else:
                            nc.gpsimd.collective_compute(
                                "AllReduce", ALU.max, replica_groups=G4,
                                ins=[gm_in.opt()], outs=[gm_out.opt()],
                            )
                        # s1f@face partials run during the allreduce; their
                        # activations are gated on gb afterwards
                        h2h = bp.tile([128, 4096], bf, tag="h2h", name="h2h")
                        chains = []
                        for ci, (ob, c0) in enumerate([(o, c) for o in range(4) for c in CHUNKS]):
                            sp1 = psA.tile([128, 512], f32, tag="sp1", bufs=5, name="sp1")
                            for kbi in range(4):
                                nc.tensor.matmul(
                                    sp1[:],
                                    S["s1fT"][:, kbi * 512 + 128 * ob: kbi * 512 + 128 * ob + 128],
                                    face_hi[:, kbi * 1024 + c0: kbi * 1024 + c0 + 512],
                                    start=(kbi == 0), stop=(kbi == 3))
                            chains.append((ob, c0, sp1))
                            if ci == 4:
                                gmg = bp.tile([128, 4], f32, tag="gmg", name="gmg")
                                nc.sync.dma_start(gmg[:], gm_out[:])
                                gmh = bp.tile([128, 4], bf, tag="gmh", name="gmh")
                                nc.vector.tensor_copy(gmh[:], gmg[:])
                                tap("d_gmg", gmg)
                                gb = bp.tile([128, 4], f32, tag="gb", name="gb")
                                for gob in range(4):
                                    gvp = psA.tile([128, 1], f32, tag="gvp", bufs=1, name="gvp")
                                    for kbi in range(4):
                                        nc.tensor.matmul(
                                            gvp[:],
                                            S["s1gT"][:, kbi * 512 + 128 * gob: kbi * 512 + 128 * gob + 128],
                                            gmh[:, kbi: kbi + 1],
                                            start=(kbi == 0), stop=(kbi == 3))
                                    nc.vector.tensor_add(gb[:, gob: gob + 1], gvp[:], S["s1b"][:, gob: gob + 1])
                        for ob, c0, sp1 in chains:
                            nc.scalar.activation(h2h[:, ob * 1024 + c0: ob * 1024 + c0 + 512],
                                                 sp1[:], AF.Prelu, bias=gb[:, ob: ob + 1], alpha=0.2)

                    h3h = bp.tile([128, 2048], bf, tag="h3h", name="h3h")
                    outsb = bp.tile([50, 1024], f32, tag="outsb", name="outsb")

                    with tc.tile_pool(name="psB", bufs=1, space="PSUM") as psB:

                        for ob in range(2):
                            for c0 in CHUNKS:
                                sp2 = psB.tile([128, 512], f32, tag="sp2", bufs=2, name="sp2")
                                for kbi in range(4):
                                    nc.tensor.matmul(
                                        sp2[:],
                                        S["s2T"][:, kbi * 256 + 128 * ob: kbi * 256 + 128 * ob + 128],
                                        h2h[:, kbi * 1024 + c0: kbi * 1024 + c0 + 512],
                                        start=(kbi == 0), stop=(kbi == 3))
                                nc.scalar.activation(h3h[:, ob * 1024 + c0: ob * 1024 + c0 + 512],
                                                     sp2[:], AF.Prelu,
                                                     bias=S["s2b"][:, ob: ob + 1], alpha=0.2)

                        for c0 in CHUNKS:
                            sp3 = psB.tile([50, 512], f32, tag="sp3", bufs=2, name="sp3")
                            for kbi in range(2):
                                nc.tensor.matmul(
                                    sp3[:],
                                    S["s3T"][:, kbi * 50: kbi * 50 + 50],
                                    h3h[:, kbi * 1024 + c0: kbi * 1024 + c0 + 512],
                                    start=(kbi == 0), stop=(kbi == 1))
                            nc.scalar.activation(outsb[:, c0: c0 + 512], sp3[:], AF.Copy)

                    tap("d_h2h", h2h)
                    nc.sync.dma_start(out_d[:], outsb[:])

            for _ in range(reps):
                emit_pipeline()

    nc.compile()
    return nc


def _prep_shared(inputs):
    g = lambda k: np.asarray(inputs[k], np.float32)
    out = {}

    def fold(wn, gn, bn):
        return g(wn) * (INV * g(gn))[:, None], g(bn)

    def emit(nm, wf):
        wT = np.ascontiguousarray(wf.T)
        if wT.shape[0] > 128:
            wT = _blockP(wT)
        out[nm] = wT.astype(BF16)

    w1, b1 = fold("conv1_w", "bn1_g", "bn1_b")
    w2, b2 = fold("conv2_w", "bn2_g", "bn2_b")
    w3, b3 = fold("conv3_w", "bn3_g", "bn3_b")
    wp1, bp1 = fold("pt1_w", "pt1_g", "pt1_b")
    wp2, bp2 = fold("pt2_w", "pt2_g", "pt2_b")
    emit("c1T", w1)
    emit("c2T", w2)
    emit("c3T", w3)
    emit("p1T", wp1)
    emit("p2T", wp2)
    fb1 = np.zeros((128, 1), np.float32)
    fb1[:64, 0] = b1
    out["fb1"] = fb1
    out["fb2"] = np.ascontiguousarray(b2[:, None])
    out["fb3"] = _blockP(b3[:, None]).astype(np.float32)
    out["fbp1"] = _blockP(bp1[:, None]).astype(np.float32)
    out["fbp2"] = _blockP(bp2[:, None]).astype(np.float32)

    for v, p in ((0, "sa1"), (1, "sa2")):
        emit(f"qkT{v}", g(p + "_qk"))
        emit(f"vwT{v}", g(p + "_vw"))
        sg, sb2 = g(p + "_g"), g(p + "_b")
        twf = g(p + "_tw") * (INV * sg)[:, None]
        emit(f"twT{v}", twf)
        out[f"vbb{v}"] = np.ascontiguousarray(
            np.broadcast_to(g(p + "_vb")[None, :], (128, 256))).astype(np.float32)
        tbfv = g(p + "_tb") * (INV * sg) + sb2
        out[f"tbf{v}"] = _blockP(tbfv[:, None]).astype(np.float32)

    cfw, cfb_ = fold("cf_w", "cf_g", "cf_b")
    emit("cfT", cfw)
    out["cfb"] = _blockP(cfb_[:, None]).astype(np.float32)
    s1w, s1b_ = fold("s1_w", "s1_g", "s1_b")
    emit("s1fT", s1w[:, :512])
    emit("s1gT", s1w[:, 512:])
    out["s1b"] = _blockP(s1b_[:, None]).astype(np.float32)
    s2w, s2b_ = fold("s2_w", "s2_g", "s2_b")
    emit("s2T", s2w)
    out["s2b"] = _blockP(s2b_[:, None]).astype(np.float32)
    emit("s3T", g("s3_w").astype(np.float32))

    WB = np.zeros((128, BF_TOT), BF16)
    WF = np.zeros((128, F32_TOT), np.float32)
    for nm, (sh, dn) in SPECS.items():
        if nm == "xT":
            continue
        a = out[nm]
        assert tuple(a.shape) == sh, (nm, a.shape, sh)
        assert (a.dtype == BF16) == (dn == "bf"), (nm, a.dtype)
        if dn == "bf":
            WB[0: sh[0], BF_OFFS[nm]: BF_OFFS[nm] + sh[1]] = a
        else:
            WF[0: sh[0], F32_OFFS[nm]: F32_OFFS[nm] + sh[1]] = a
    return {"WB": WB, "WF": WF}


def _get_nc(debug=False, reps=1):
    key = ("nc_dbg" if debug else "nc") + str(reps)
    if key not in _CACHE:
        _CACHE[key] = _build(debug, reps)
    return _CACHE[key]


def _in_maps(inputs):
    base = _prep_shared(inputs)
    x = np.asarray(inputs["x"], np.float32)
    maps = []
    for c in range(8):
        b, j = c // 4, c % 4
        xT = np.ascontiguousarray(x[b, 1024 * j: 1024 * (j + 1), :].T).astype(BF16)
        m = dict(base)
        m["xT"] = xT
        maps.append(m)
    return maps


def _assemble(res):
    full = np.empty((2, 4096, 50), np.float32)
    for c in range(8):
        b, j = c // 4, c % 4
        full[b, 1024 * j: 1024 * (j + 1), :] = np.asarray(res.results[c]["out"], np.float32).T
    return full


def kernel(**inputs):
    from concourse.bass_utils import run_bass_kernel_spmd
    nc = _get_nc()
    res = run_bass_kernel_spmd(nc, _in_maps(inputs), core_ids=list(range(8)))
    return _assemble(res)


def run_traced(inputs, trace_cores=None):
    from concourse.bass_utils import run_bass_kernel_spmd
    nc = _get_nc()
    res = run_bass_kernel_spmd(
        nc, _in_maps(inputs), core_ids=list(range(8)),
        trace=True, trace_cores=trace_cores or [0],
    )
    return _assemble(res), res


def run_debug(inputs):
    from concourse.bass_utils import run_bass_kernel_spmd
    nc = _get_nc(debug=True)
    res = run_bass_kernel_spmd(nc, _in_maps(inputs), core_ids=list(range(8)))
    return res


def measure_hw_ns(inputs, M=64, reps=1):
    import time
    import jax
    from jax.sharding import Mesh, PartitionSpec, NamedSharding
    from jax.experimental.shard_map import shard_map
    from concourse import mybir
    from concourse.bass2jax import _bass_exec_p, install_neuronx_cc_hook, partition_id_tensor

    nc = _get_nc(reps=reps)
    install_neuronx_cc_hook()
    in_maps = _in_maps(inputs)
    partition_name = nc.partition_id_tensor.name if nc.partition_id_tensor else None
    in_names, out_names, out_avals, zero_outs = [], [], [], []
    for alloc in nc.m.functions[0].allocations:
        if not isinstance(alloc, mybir.MemoryLocationSet):
            continue
        name = alloc.memorylocations[0].name
        if alloc.kind == "ExternalInput":
            if name != partition_name:
                in_names.append(name)
        elif alloc.kind == "ExternalOutput":
            out_names.append(name)
            shape = tuple(alloc.tensor_shape)
            dtype = mybir.dt.np(alloc.dtype)
            out_avals.append(jax.core.ShapedArray(shape, dtype))
            zero_outs.append(np.zeros(shape, dtype))
    n_params = len(in_names)
    in_names_all = in_names + out_names
    if partition_name is not None:
        in_names_all.append(partition_name)

    def _body(*args):
        operands = list(args)
        if partition_name is not None:
            operands.append(partition_id_tensor())
        outs = _bass_exec_p.bind(
            *operands, out_avals=tuple(out_avals), in_names=tuple(in_names_all),
            out_names=tuple(out_names), lowering_input_output_aliases=(),
            sim_require_finite=True, sim_require_nnan=True, nc=nc)
        return tuple(outs)

    devices = jax.devices()[:8]
    mesh = Mesh(np.asarray(devices), ("core",))
    spec = PartitionSpec("core")
    fn = jax.jit(
        shard_map(_body, mesh=mesh, in_specs=(spec,) * (n_params + len(out_avals)),
                  out_specs=(spec,) * len(out_avals), check_rep=False),
        keep_unused=True)
    per_core = [[np.asarray(m[name]) for name in in_names] for m in in_maps]
    concat_in = [np.concatenate([per_core[c][i] for c in range(8)], axis=0)
                 for i in range(n_params)]
    concat_zeros = [np.zeros((8 * zz.shape[0], *zz.shape[1:]), zz.dtype) for zz in zero_outs]
    sh = NamedSharding(mesh, spec)
    dev_in = [jax.device_put(a, sh) for a in concat_in]
    dev_zero = [jax.device_put(a, sh) for a in concat_zeros]
    o = fn(*dev_in, *dev_zero)
    jax.block_until_ready(o)
    t0 = time.perf_counter()
    outs = [fn(*dev_in, *dev_zero) for _ in range(M)]
    jax.block_until_ready(outs)
    t1 = time.perf_counter()
    return (t1 - t0) / M * 1e9



# revision 44
# speedup vs baseline: 1.3087x; 1.3087x over previous
import os
import sys

os.environ.setdefault("JAX_PLATFORMS", "")
sys.path.insert(0, "/opt/trn_rl_repo")

import numpy as np
import ml_dtypes

BF16 = ml_dtypes.bfloat16
INV = np.float32(1.0 / np.sqrt(1.0 + 1e-5))
G8 = [[0, 1, 2, 3, 4, 5, 6, 7]]
G4 = [[0, 1, 2, 3], [4, 5, 6, 7]]
AG4 = True
FP8 = True
NOCOLL = os.environ.get("V2_NOCOLL", "0") == "1"
NLOC = 1024
CHUNKS = (0, 512)

_CACHE = {}


def _blockP(a):
    a = np.ascontiguousarray(a)
    R, C = a.shape
    assert R % 128 == 0, (R, C)
    nb = R // 128
    return np.ascontiguousarray(a.reshape(nb, 128, C).transpose(1, 0, 2).reshape(128, nb * C))


# name -> (shape, "bf"/"f32"/"u32")
SPECS = {
    "xT": ((3, 1024), "bf"),
    "c1T": ((3, 64), "bf"),
    "c2T": ((64, 128), "bf"),
    "c3T": ((128, 256), "bf"),
    "p1T": ((128, 512), "bf"),
    "p2T": ((128, 512), "bf"),
    "fb1": ((128, 1), "f32"), "fb2": ((128, 1), "f32"), "fb3": ((128, 2), "f32"),
    "fbp1": ((128, 2), "f32"), "fbp2": ((128, 2), "f32"),
    "gqT0": ((128, 512), "bf"), "gqT1": ((128, 512), "bf"),
    "vwT0": ((128, 512), "bf"), "vwT1": ((128, 512), "bf"),
    "twT0": ((128, 512), "bf"), "twT1": ((128, 512), "bf"),
    "vbb0": ((128, 256), "f32"), "vbb1": ((128, 256), "f32"),
    "tbf0": ((128, 2), "f32"), "tbf1": ((128, 2), "f32"),
    "cfT": ((128, 4096), "bf"), "cfb": ((128, 4), "f32"),
    "s1fT": ((128, 2048), "bf"), "s1gT": ((128, 2048), "bf"), "s1b": ((128, 4), "f32"),
    "s2T": ((128, 1024), "bf"), "s2b": ((128, 2), "f32"),
    "s3T": ((128, 100), "bf"),
}

# pack every weight (everything but per-core xT) into two mega input
# tensors so each call ships 3 PJRT buffers instead of ~34
BF_OFFS, F32_OFFS = {}, {}
_bo = _fo = 0
for _nm, (_sh, _dn) in SPECS.items():
    if _nm == "xT":
        continue
    if _dn == "bf":
        BF_OFFS[_nm] = _bo
        _bo += _sh[1]
    else:
        F32_OFFS[_nm] = _fo
        _fo += _sh[1]
BF_TOT, F32_TOT = _bo, _fo


DBG_SPECS = {
    "d_front": ((128, 2048), "f32"),
    "d_xq0": ((128, 1024), "bf"),
    "d_xqf": ((128, 4096), "bf"),
    "d_xvtf": ((128, 8192), "bf"),
    "d_rsg": ((128, 32), "f32"),
    "d_cs": ((1, 1024), "f32"),
    "d_xrh": ((128, 2048), "bf"),
    "d_xm1": ((128, 2048), "f32"),
    "d_xm4": ((128, 2048), "f32"),
    "d_gmg": ((128, 4), "f32"),
    "d_h2h": ((128, 4096), "bf"),
}


def _build(debug=False, reps=1):
    from concourse import tile, bacc, mybir
    from concourse.bass import ds

    dt = mybir.dt
    AF = mybir.ActivationFunctionType
    AX = mybir.AxisListType
    ALU = mybir.AluOpType
    bf, f32 = dt.bfloat16, dt.float32

    nc = bacc.Bacc("TRN2", target_bir_lowering=False, debug=False, num_devices=8)

    WB = nc.dram_tensor("WB", [128, BF_TOT], bf, kind="ExternalInput")
    WF = nc.dram_tensor("WF", [128, F32_TOT], f32, kind="ExternalInput")
    XT = nc.dram_tensor("xT", [3, 1024], bf, kind="ExternalInput")
    out_d = nc.dram_tensor("out", [50, 1024], f32, kind="ExternalOutput")
    DBG = {}
    if debug:
        for nm, (sh, dn) in DBG_SPECS.items():
            DBG[nm] = nc.dram_tensor(nm, list(sh), bf if dn == "bf" else f32,
                                     kind="ExternalOutput")

    def tap(nm, t):
        if debug and nm in DBG:
            nc.gpsimd.dma_start(DBG[nm][:], t[:])

    with tile.TileContext(nc) as tc:
        with (
            tc.tile_pool(name="pers", bufs=1) as pers,
            tc.tile_pool(name="dramp", bufs=1, space="DRAM") as dramp,
        ):
            def sload(pool, nm, eng=None):
                sh, dn = SPECS[nm]
                dty = {"bf": bf, "f32": f32}[dn]
                t = pool.tile(list(sh), dty, tag=nm, name=nm + "_sb")
                if nm == "xT":
                    src = XT[:]
                elif dn == "bf":
                    src = WB[:][0: sh[0], BF_OFFS[nm]: BF_OFFS[nm] + sh[1]]
                else:
                    src = WF[:][0: sh[0], F32_OFFS[nm]: F32_OFFS[nm] + sh[1]]
                (eng or nc.sync).dma_start(t[:], src)
                return t

            S = {}
            # SA weights on gpsimd queue, back weights on scalar queue
            # (front weights go on sync in fp below)
            for nm in ("gqT0", "gqT1", "vwT0", "vwT1", "twT0", "twT1",
                       "vbb0", "vbb1", "tbf0", "tbf1"):
                S[nm] = sload(pers, nm, eng=nc.gpsimd)
            for nm in ("cfT", "cfb", "s1fT", "s1gT", "s1b", "s2T", "s2b", "s3T"):
                S[nm] = sload(pers, nm, eng=nc.scalar)


            xm = pers.tile([128, 2048], f32, tag="xm")
            slot_h = [pers.tile([128, 2048], bf, tag=f"sh{s}", name=f"sh{s}") for s in range(5)]
            ones1 = pers.tile([1, 128], f32, tag="ones1")
            nc.gpsimd.memset(ones1[:], 1.0)

            agq_in = dramp.tile([128, 2048], dt.float8e4, tag="agq_in")
            agq_out = dramp.tile([512, 2048], dt.float8e4, tag="agq_out")
            agv_in = dramp.tile([128, 2048], dt.float8e4 if FP8 else bf, tag="agv_in")
            agv_out = dramp.tile([512 if AG4 else 1024, 2048], dt.float8e4 if FP8 else bf, tag="agv_out")
            ar_in = dramp.tile([128, 16], f32, tag="ar_in")
            ar_out = dramp.tile([128, 16], f32, tag="ar_out")
            ar_in2 = dramp.tile([128, 16], f32, tag="ar_in2")
            ar_out2 = dramp.tile([128, 16], f32, tag="ar_out2")
            gm_in = dramp.tile([128, 4], f32, tag="gm_in")
            gm_out = dramp.tile([128, 4], f32, tag="gm_out")

            import itertools
            _regctr = itertools.count()

            def _pair_ap(t, gp, c0):
                return t[:].rearrange("p (g n) -> p g n", n=1024)[:, 2 * gp: 2 * gp + 2, c0: c0 + 512]

            def _pair_lhs(t, gp, cb):
                return t[:].rearrange("p (g c) -> p g c", c=256)[:, 2 * gp: 2 * gp + 2, cb * 128: cb * 128 + 128]

            def _pair_rb(t, gp):
                return t[:].rearrange("p (g o) -> p g o", o=16)[:, 2 * gp: 2 * gp + 2, 0:1]

            def emit_pipeline():
                # ---------------- front chain ----------------
                with (
                    tc.tile_pool(name="fp", bufs=1) as fp,
                    tc.tile_pool(name="fps", bufs=1, space="PSUM") as fps,
                ):
                    for nm in ("xT", "c1T", "c2T", "c3T", "p1T", "p2T",
                               "fb1", "fb2", "fb3", "fbp1", "fbp2"):
                        S[nm] = sload(fp, nm)

                    chain = [
                        ("c1T", 1, 64, "fb1"),
                        ("c2T", 1, 128, "fb2"),
                        ("c3T", 1, 256, "fb3"),
                        ("p1T", 2, 256, "fbp1"),
                        ("p2T", 2, 256, "fbp2"),
                    ]
                    cur = S["xT"]
                    for li, (wn, kb, Cout, bn) in enumerate(chain):
                        w, bt = S[wn], S[bn]
                        nob = (Cout + 127) // 128
                        P_out = min(Cout, 128)
                        last = li == len(chain) - 1
                        if last:
                            of = xm
                        else:
                            of = fp.tile([P_out, nob * 1024], f32, tag=f"hf{li}", name=f"hf{li}")
                        for ob in range(nob):
                            Mob = min(128, Cout - 128 * ob)
                            ps = fps.tile([Mob, 1024], f32, tag="fps_t", bufs=2, name="fps_t")
                            for c0 in CHUNKS:
                                for kbi in range(kb):
                                    nc.tensor.matmul(
                                        ps[:, c0:c0 + 512],
                                        w[:, kbi * Cout + 128 * ob: kbi * Cout + 128 * ob + Mob],
                                        cur[:, kbi * 1024 + c0: kbi * 1024 + c0 + 512],
                                        start=(kbi == 0), stop=(kbi == kb - 1))
                            nc.scalar.activation(
                                of[0:Mob, ob * 1024: ob * 1024 + 1024], ps[:],
                                AF.Relu, bias=bt[0:Mob, ob: ob + 1],
                            )
                        if last:
                            nh = slot_h[0]
                        else:
                            nh = fp.tile([P_out, nob * 1024], bf, tag=f"hh{li}", name=f"hh{li}")
                        W = nob * 1024
                        nc.vector.tensor_copy(nh[0:P_out, 0:W], of[0:P_out, 0:W])
                        cur = nh
                tap("d_front", xm)

                # ---------------- 4 SA layers ----------------
                with tc.tile_pool(name="sap", bufs=1) as sp:
                    for L in range(4):
                        v = 0 if L == 0 else 1
                        ih = slot_h[L]
                        oh = slot_h[L + 1]
                        qh = S[f"gqT{v}"]
                        vh = S[f"vwT{v}"]
                        th = S[f"twT{v}"]
                        vbb, tbt = S[f"vbb{v}"], S[f"tbf{v}"]

                        hq8 = sp.tile([128, 2048], dt.float8e4, tag="hq8", name="hq8")
                        z_loc = sp.tile([128, 2048], dt.float8e4, tag="z_loc", name="z_loc")
                        xvt_loc = sp.tile([128, 2048], dt.float8e4 if FP8 else bf, tag="xvt_loc", name="xvt_loc")
                        xq_full = sp.tile([128, 8192], dt.float8e4, tag="xq_full", name="xq_full")
                        xvt_full = sp.tile([128, 8192], dt.float8e4 if FP8 else bf, tag="xvt_full", name="xvt_full")
                        P_sb = sp.tile([128, 32 * 1024], dt.float8e4 if FP8 else bf, tag="P_sb", name="P_sb")
                        rs_sb = sp.tile([128, 32], f32, tag="rs_sb", name="rs_sb")
                        rs_g = sp.tile([128, 32], f32, tag="rs_g", name="rs_g")
                        r_f = sp.tile([128, 32], f32, tag="r_f", name="r_f")
                        r_b = sp.tile([128, 32], bf, tag="r_b", name="r_b")
                        A_sb = sp.tile([128, 2048], f32, tag="A_sb", name="A_sb")
                        xr_hi = sp.tile([128, 2048], bf, tag="xr_hi", name="xr_hi")
                        if FP8:
                            xvt_f8 = sp.tile([128, 8192], dt.float8e4, tag="xvt_f8", name="xvt_f8")
                            r_b8 = sp.tile([128, 512], dt.float8e4, tag="r_b8", name="r_b8")
                        icb = sp.tile([128, 1024], f32, tag="icb", name="icb")
                        cs_row = sp.tile([1, 1024], f32, tag="cs_row", name="cs_row")
                        ic_row = sp.tile([1, 1024], f32, tag="ic_row", name="ic_row")

                        # phase 0: ship h (fp8) via AG1 asap; z = (1024*G) @ h
                        # locally so energy runs fp8 DoubleRow over 256 ch
                        with tc.tile_pool(name="ps0", bufs=1, space="PSUM") as ps0:
                            nc.scalar.activation(hq8[:], ih[:], AF.Copy)
                            nc.sync.dma_start(agq_in[:], hq8[:])
                            if NOCOLL:
                                for k in range(4):
                                    nc.gpsimd.dma_start(agq_out[k * 128:(k + 1) * 128, :], agq_in[:])
                            else:
                                nc.gpsimd.collective_compute(
                                    "AllGather", ALU.bypass, replica_groups=G4 if AG4 else G8,
                                    ins=[agq_in.opt()], outs=[agq_out.opt()],
                                )
                            for ob in range(2):
                                zps = ps0.tile([128, 1024], f32, tag="qps", bufs=1, name="zps")
                                for c0 in CHUNKS:
                                    for kbi in range(2):
                                        nc.tensor.matmul(
                                            zps[:, c0: c0 + 512],
                                            qh[:, kbi * 256 + 128 * ob: kbi * 256 + 128 * ob + 128],
                                            ih[:, kbi * 1024 + c0: kbi * 1024 + c0 + 512],
                                            start=(kbi == 0), stop=(kbi == 1))
                                nc.scalar.activation(z_loc[:, ob * 1024: ob * 1024 + 1024],
                                                     zps[:], AF.Copy)
                            # xvt, then AG2
                            for nb in range(8):
                                vps = ps0.tile([128, 256], f32, tag="vps", bufs=2, name="vps")
                                for kbi in range(2):
                                    nc.tensor.matmul(
                                        vps[:], ih[:, kbi * 1024 + nb * 128: kbi * 1024 + nb * 128 + 128],
                                        vh[:, kbi * 256: kbi * 256 + 256],
                                        start=(kbi == 0), stop=(kbi == 1))
                                nc.vector.tensor_add(xvt_loc[:, nb * 256: nb * 256 + 256], vps[:], vbb[:])
                            nc.sync.dma_start(agv_in[:], xvt_loc[:])
                            if NOCOLL:
                                for k in range(4):
                                    nc.gpsimd.dma_start(agv_out[k * 128:(k + 1) * 128, :], agv_in[:])
                            else:
                                nc.gpsimd.collective_compute(
                                    "AllGather", ALU.bypass, replica_groups=G4 if AG4 else G8,
                                    ins=[agv_in.opt()], outs=[agv_out.opt()],
                                )
                            # A = tw @ x while AG1 in flight
                            for ob in range(2):
                                aps = ps0.tile([128, 1024], f32, tag="aps", bufs=2, name="aps")
                                for c0 in CHUNKS:
                                    for kbi in range(2):
                                        nc.tensor.matmul(
                                            aps[:, c0: c0 + 512],
                                            th[:, kbi * 256 + 128 * ob: kbi * 256 + 128 * ob + 128],
                                            ih[:, kbi * 1024 + c0: kbi * 1024 + c0 + 512],
                                            start=(kbi == 0), stop=(kbi == 1))
                                nc.scalar.activation(A_sb[:, ob * 1024: ob * 1024 + 1024],
                                                     aps[:], AF.Copy)

                        # unstage h_full (AG1: [4k x 128p, 2c x 1024j] ->
                        # [128p, 2c, 4k*1024j]) then xvt (AG2)
                        nc.sync.dma_start(
                            xq_full[:].rearrange("p (c k j) -> p c k j", c=2, k=4),
                            agq_out[:].rearrange("(k p) (c j) -> p c k j", p=128, c=2))
                        nc.sync.dma_start(xvt_full[:].rearrange("p (k j) -> p k j", k=4),
                                          agv_out[:].rearrange("(k p) j -> p k j", p=128))

                        # phase 2: energy + exp(P); rowsums ride on accum_out
                        hf3 = xq_full[:].rearrange("p (c n) -> p c n", n=4096)
                        z3 = z_loc[:].rearrange("p (c j) -> p c j", j=1024)
                        with tc.tile_pool(name="ps2", bufs=1, space="PSUM") as ps2:
                            for gp in range(16):
                                eps = ps2.tile([128, 2048], f32, tag="eps", bufs=2, name="eps")
                                for gi in range(2):
                                    g = 2 * gp + gi
                                    for c0 in CHUNKS:
                                        nc.tensor.matmul(eps[:, gi * 1024 + c0: gi * 1024 + c0 + 512],
                                                         hf3[:, :, g * 128: g * 128 + 128],
                                                         z3[:, :, c0: c0 + 512],
                                                         start=True, stop=True,
                                                         perf_mode=mybir.MatmulPerfMode.DoubleRow)
                                for gi in range(2):
                                    g = 2 * gp + gi
                                    nc.scalar.activation(P_sb[:, g * 1024: g * 1024 + 1024],
                                                         eps[:, gi * 1024: gi * 1024 + 1024],
                                                         AF.Exp, scale=1.0 / 1024.0,
                                                         accum_out=rs_sb[:, g: g + 1])
                                if gp == 7:
                                    nc.sync.dma_start(ar_in[:], rs_sb[:, 0:16])
                                    if NOCOLL:
                                        nc.gpsimd.dma_start(ar_out[:], ar_in[:])
                                    else:
                                        nc.gpsimd.collective_compute(
                                            "AllReduce", ALU.add, replica_groups=G4,
                                            ins=[ar_in.opt()], outs=[ar_out.opt()],
                                        )

                        # phase 3: second-half rowsum allreduce (first half ran under exp)
                        nc.sync.dma_start(ar_in2[:], rs_sb[:, 16:32])
                        if NOCOLL:
                            nc.gpsimd.dma_start(ar_out2[:], ar_in2[:])
                        else:
                            nc.gpsimd.collective_compute(
                                "AllReduce", ALU.add, replica_groups=G4,
                                ins=[ar_in2.opt()], outs=[ar_out2.opt()],
                            )
                        nc.sync.dma_start(rs_g[:, 0:16], ar_out[:])
                        nc.sync.dma_start(rs_g[:, 16:32], ar_out2[:])
                        if FP8:
                            nc.scalar.mul(rs_g[:], rs_g[:], 1.0 / 1024.0)
                        nc.vector.reciprocal(r_f[:], rs_g[:])
                        if FP8:
                            nc.vector.tensor_copy(
                                r_b8[:].rearrange("p (g o) -> p g o", o=16)[:, :, 0:1],
                                r_f[:].unsqueeze(2))
                        else:
                            nc.vector.tensor_copy(r_b[:], r_f[:])

                        with tc.tile_pool(name="ps3", bufs=1, space="PSUM") as ps3:
                            # colsum (needs r_b + P only) runs while vector scales xvt
                            for ci, c0 in enumerate(CHUNKS):
                                csp = ps3.tile([1, 512], f32, tag=f"csp{ci}", name=f"csp{ci}")
                                if FP8:
                                    for gp in range(16):
                                        nc.tensor.matmul(
                                            csp[:], _pair_rb(r_b8, gp),
                                            _pair_ap(P_sb, gp, c0),
                                            start=(gp == 0), stop=(gp == 15),
                                            perf_mode=mybir.MatmulPerfMode.DoubleRow)
                                else:
                                    for g in range(32):
                                        nc.tensor.matmul(csp[:], r_b[:, g: g + 1],
                                                         P_sb[:, g * 1024 + c0: g * 1024 + c0 + 512],
                                                         start=(g == 0), stop=(g == 31))
                                nc.vector.tensor_scalar_add(cs_row[:, c0: c0 + 512], csp[:], 1e-9)
                            nc.vector.reciprocal(ic_row[:], cs_row[:])
                            for g in range(32):
                                if FP8:
                                    nc.vector.tensor_scalar_mul(
                                        xvt_f8[:, g * 256: (g + 1) * 256],
                                        xvt_full[:, g * 256: (g + 1) * 256], r_f[:, g: g + 1])
                                else:
                                    nc.vector.tensor_scalar_mul(
                                        xvt_full[:, g * 256: (g + 1) * 256],
                                        xvt_full[:, g * 256: (g + 1) * 256], r_f[:, g: g + 1])
                            for c0 in CHUNKS:
                                ibp = ps3.tile([128, 512], f32, tag="ibp", bufs=2, name="ibp")
                                nc.tensor.matmul(ibp[:], ones1[:], ic_row[:, c0: c0 + 512], start=True, stop=True)
                                nc.scalar.activation(icb[:, c0: c0 + 512], ibp[:], AF.Copy)
                            for cb in range(2):
                                for c0 in CHUNKS:
                                    xrp = ps3.tile([128, 512], f32, tag="xrp", bufs=2, name="xrp")
                                    if FP8:
                                        for gp in range(16):
                                            nc.tensor.matmul(
                                                xrp[:],
                                                _pair_lhs(xvt_f8, gp, cb),
                                                _pair_ap(P_sb, gp, c0),
                                                start=(gp == 0), stop=(gp == 15),
                                                perf_mode=mybir.MatmulPerfMode.DoubleRow)
                                    else:
                                        for g in range(32):
                                            nc.tensor.matmul(
                                                xrp[:],
                                                xvt_full[:, g * 256 + cb * 128: g * 256 + cb * 128 + 128],
                                                P_sb[:, g * 1024 + c0: g * 1024 + c0 + 512],
                                                start=(g == 0), stop=(g == 31))
                                    sl = slice(cb * 1024 + c0, cb * 1024 + c0 + 512)
                                    nc.scalar.activation(xr_hi[:, sl], xrp[:], AF.Copy)

                        if L == 0:
                            tap("d_rsg", rs_g)
                            tap("d_cs", cs_row)
                            tap("d_xrh", xr_hi)
                            tap("d_xvtf", xvt_full)

                        # phase 4: B = tw@xr, y = relu(A - B*icb + tbf), resid add
                        with tc.tile_pool(name="ps4", bufs=1, space="PSUM") as ps4:
                            for ob in range(2):
                                for c0 in CHUNKS:
                                    bps = ps4.tile([128, 512], f32, tag="bps", bufs=2, name="bps")
                                    for kbi in range(2):
                                        nc.tensor.matmul(
                                            bps[:], th[:, kbi * 256 + 128 * ob: kbi * 256 + 128 * ob + 128],
                                            xr_hi[:, kbi * 1024 + c0: kbi * 1024 + c0 + 512],
                                            start=(kbi == 0), stop=(kbi == 1))
                                    osl = slice(ob * 1024 + c0, ob * 1024 + c0 + 512)
                                    tmp = sp.tile([128, 512], f32, tag="scr", bufs=2, name="tmp")
                                    nc.vector.tensor_mul(tmp[:], bps[:], icb[:, c0: c0 + 512])
                                    nc.vector.tensor_sub(A_sb[:, osl], A_sb[:, osl], tmp[:])
                                    yv = sp.tile([128, 512], f32, tag="scr", bufs=2, name="yv")
                                    nc.scalar.activation(yv[:], A_sb[:, osl], AF.Relu, bias=tbt[:, ob: ob + 1])
                                    if L == 3:
                                        nc.vector.tensor_add(oh[:, osl], xm[:, osl], yv[:])
                                    else:
                                        nc.vector.tensor_add(xm[:, osl], xm[:, osl], yv[:])
                                        nc.vector.tensor_copy(oh[:, osl], xm[:, osl])
                        if L == 0:
                            tap("d_xm1", xm)
                        if L == 3:
                            tap("d_xm4", xm)

                # ---------------- back end ----------------
                with tc.tile_pool(name="bp", bufs=1) as bp:
                    face_hi = bp.tile([128, 4096], bf, tag="face_hi", name="face_hi")
                    gml = bp.tile([128, 4], f32, tag="gml", name="gml")

                    with tc.tile_pool(name="psA", bufs=1, space="PSUM") as psA:
                        for ob in range(4):
                            for c0 in CHUNKS:
                                fpt = psA.tile([128, 512], f32, tag="fpsb", bufs=2, name="fpt")
                                for sk in range(8):
                                    s, cb = 1 + sk // 2, sk % 2
                                    nc.tensor.matmul(
                                        fpt[:],
                                        S["cfT"][:, sk * 512 + 128 * ob: sk * 512 + 128 * ob + 128],
                                        slot_h[s][:, cb * 1024 + c0: cb * 1024 + c0 + 512],
                                        start=(sk == 0), stop=(sk == 7))
                                nc.scalar.activation(face_hi[:, ob * 1024 + c0: ob * 1024 + c0 + 512],
                                                     fpt[:], AF.Prelu,
                                                     bias=S["cfb"][:, ob: ob + 1], alpha=0.2)
                            nc.vector.tensor_reduce(gml[:, ob: ob + 1],
                                                    face_hi[:, ob * 1024: (ob + 1) * 1024],
                                                    axis=AX.X, op=ALU.max)

                        nc.sync.dma_start(gm_in[:], gml[:])
                        if NOCOLL:
                            nc.gpsimd.dma_start(gm_out[:], gm_in[:])
                        else:
                            nc.gpsimd.collective_compute(
                                "AllReduce", ALU.max, replica_groups=G4,
                                ins=[gm_in.opt()], outs=[gm_out.opt()],
                            )
                        # s1f@face partials run during the allreduce; their
                        # activations are gated on gb afterwards
                        h2h = bp.tile([128, 4096], bf, tag="h2h", name="h2h")
                        chains = []
                        for ci, (ob, c0) in enumerate([(o, c) for o in range(4) for c in CHUNKS]):
                            sp1 = psA.tile([128, 512], f32, tag="sp1", bufs=5, name="sp1")
                            for kbi in range(4):
                                nc.tensor.matmul(
                                    sp1[:],
                                    S["s1fT"][:, kbi * 512 + 128 * ob: kbi * 512 + 128 * ob + 128],
                                    face_hi[:, kbi * 1024 + c0: kbi * 1024 + c0 + 512],
                                    start=(kbi == 0), stop=(kbi == 3))
                            chains.append((ob, c0, sp1))
                            if ci == 4:
                                gmg = bp.tile([128, 4], f32, tag="gmg", name="gmg")
                                nc.sync.dma_start(gmg[:], gm_out[:])
                                gmh = bp.tile([128, 4], bf, tag="gmh", name="gmh")
                                nc.vector.tensor_copy(gmh[:], gmg[:])
                                tap("d_gmg", gmg)
                                gb = bp.tile([128, 4], f32, tag="gb", name="gb")
                                for gob in range(4):
                                    gvp = psA.tile([128, 1], f32, tag="gvp", bufs=1, name="gvp")
                                    for kbi in range(4):
                                        nc.tensor.matmul(
                                            gvp[:],
                                            S["s1gT"][:, kbi * 512 + 128 * gob: kbi * 512 + 128 * gob + 128],
                                            gmh[:, kbi: kbi + 1],
                                            start=(kbi == 0), stop=(kbi == 3))
                                    nc.vector.tensor_add(gb[:, gob: gob + 1], gvp[:], S["s1b"][:, gob: gob + 1])
                        for ob, c0, sp1 in chains:
                            nc.scalar.activation(h2h[:, ob * 1024 + c0: ob * 1024 + c0 + 512],
                                                 sp1[:], AF.Prelu, bias=gb[:, ob: ob + 1], alpha=0.2)

                    h3h = bp.tile([128, 2048], bf, tag="h3h", name="h3h")
                    outsb = bp.tile([50, 1024], f32, tag="outsb", name="outsb")

                    with tc.tile_pool(name="psB", bufs=1, space="PSUM") as psB:

                        for ob in range(2):
                            for c0 in CHUNKS:
                                sp2 = psB.tile([128, 512], f32, tag="sp2", bufs=2, name="sp2")
                                for kbi in range(4):
                                    nc.tensor.matmul(
                                        sp2[:],
                                        S["s2T"][:, kbi * 256 + 128 * ob: kbi * 256 + 128 * ob + 128],
                                        h2h[:, kbi * 1024 + c0: kbi * 1024 + c0 + 512],
                                        start=(kbi == 0), stop=(kbi == 3))
                                nc.scalar.activation(h3h[:, ob * 1024 + c0: ob * 1024 + c0 + 512],
                                                     sp2[:], AF.Prelu,
                                                     bias=S["s2b"][:, ob: ob + 1], alpha=0.2)

                        for c0 in CHUNKS:
                            sp3 = psB.tile([50, 512], f32, tag="sp3", bufs=2, name="sp3")
                            for kbi in range(2):
                                nc.tensor.matmul(
                                    sp3[:],
                                    S["s3T"][:, kbi * 50: kbi * 50 + 50],
                                    h3h[:, kbi * 1024 + c0: kbi * 1024 + c0 + 512],
                                    start=(kbi == 0), stop=(kbi == 1))
                            nc.scalar.activation(outsb[:, c0: c0 + 512], sp3[:], AF.Copy)

                    tap("d_h2h", h2h)
                    nc.sync.dma_start(out_d[:], outsb[:])

            for _ in range(reps):
                emit_pipeline()

    nc.compile()
    return nc


def _prep_shared(inputs):
    g = lambda k: np.asarray(inputs[k], np.float32)
    out = {}

    def fold(wn, gn, bn):
        return g(wn) * (INV * g(gn))[:, None], g(bn)

    def emit(nm, wf):
        wT = np.ascontiguousarray(wf.T)
        if wT.shape[0] > 128:
            wT = _blockP(wT)
        out[nm] = wT.astype(BF16)

    w1, b1 = fold("conv1_w", "bn1_g", "bn1_b")
    w2, b2 = fold("conv2_w", "bn2_g", "bn2_b")
    w3, b3 = fold("conv3_w", "bn3_g", "bn3_b")
    wp1, bp1 = fold("pt1_w", "pt1_g", "pt1_b")
    wp2, bp2 = fold("pt2_w", "pt2_g", "pt2_b")
    emit("c1T", w1)
    emit("c2T", w2)
    emit("c3T", w3)
    emit("p1T", wp1)
    emit("p2T", wp2)
    fb1 = np.zeros((128, 1), np.float32)
    fb1[:64, 0] = b1
    out["fb1"] = fb1
    out["fb2"] = np.ascontiguousarray(b2[:, None])
    out["fb3"] = _blockP(b3[:, None]).astype(np.float32)
    out["fbp1"] = _blockP(bp1[:, None]).astype(np.float32)
    out["fbp2"] = _blockP(bp2[:, None]).astype(np.float32)

    for v, p in ((0, "sa1"), (1, "sa2")):
        qk = g(p + "_qk")
        emit(f"gqT{v}", np.float32(1024.0) * (qk.T @ qk))
        emit(f"vwT{v}", g(p + "_vw"))
        sg, sb2 = g(p + "_g"), g(p + "_b")
        twf = g(p + "_tw") * (INV * sg)[:, None]
        emit(f"twT{v}", twf)
        out[f"vbb{v}"] = np.ascontiguousarray(
            np.broadcast_to(g(p + "_vb")[None, :], (128, 256))).astype(np.float32)
        tbfv = g(p + "_tb") * (INV * sg) + sb2
        out[f"tbf{v}"] = _blockP(tbfv[:, None]).astype(np.float32)

    cfw, cfb_ = fold("cf_w", "cf_g", "cf_b")
    emit("cfT", cfw)
    out["cfb"] = _blockP(cfb_[:, None]).astype(np.float32)
    s1w, s1b_ = fold("s1_w", "s1_g", "s1_b")
    emit("s1fT", s1w[:, :512])
    emit("s1gT", s1w[:, 512:])
    out["s1b"] = _blockP(s1b_[:, None]).astype(np.float32)
    s2w, s2b_ = fold("s2_w", "s2_g", "s2_b")
    emit("s2T", s2w)
    out["s2b"] = _blockP(s2b_[:, None]).astype(np.float32)
    emit("s3T", g("s3_w").astype(np.float32))

    WB = np.zeros((128, BF_TOT), BF16)
    WF = np.zeros((128, F32_TOT), np.float32)
    for nm, (sh, dn) in SPECS.items():
        if nm == "xT":
            continue
        a = out[nm]
        assert tuple(a.shape) == sh, (nm, a.shape, sh)
        assert (a.dtype == BF16) == (dn == "bf"), (nm, a.dtype)
        if dn == "bf":
            WB[0: sh[0], BF_OFFS[nm]: BF_OFFS[nm] + sh[1]] = a
        else:
            WF[0: sh[0], F32_OFFS[nm]: F32_OFFS[nm] + sh[1]] = a
    return {"WB": WB, "WF": WF}


def _get_nc(debug=False, reps=1):
    key = ("nc_dbg" if debug else "nc") + str(reps)
    if key not in _CACHE:
        _CACHE[key] = _build(debug, reps)
    return _CACHE[key]


def _in_maps(inputs):
    base = _prep_shared(inputs)
    x = np.asarray(inputs["x"], np.float32)
    maps = []
    for c in range(8):
        b, j = c // 4, c % 4
        xT = np.ascontiguousarray(x[b, 1024 * j: 1024 * (j + 1), :].T).astype(BF16)
        m = dict(base)
        m["xT"] = xT
        maps.append(m)
    return maps


def _assemble(res):
    full = np.empty((2, 4096, 50), np.float32)
    for c in range(8):
        b, j = c // 4, c % 4
        full[b, 1024 * j: 1024 * (j + 1), :] = np.asarray(res.results[c]["out"], np.float32).T
    return full


def kernel(**inputs):
    from concourse.bass_utils import run_bass_kernel_spmd
    nc = _get_nc()
    res = run_bass_kernel_spmd(nc, _in_maps(inputs), core_ids=list(range(8)))
    return _assemble(res)


def run_traced(inputs, trace_cores=None):
    from concourse.bass_utils import run_bass_kernel_spmd
    nc = _get_nc()
    res = run_bass_kernel_spmd(
        nc, _in_maps(inputs), core_ids=list(range(8)),
        trace=True, trace_cores=trace_cores or [0],
    )
    return _assemble(res), res


def run_debug(inputs):
    from concourse.bass_utils import run_bass_kernel_spmd
    nc = _get_nc(debug=True)
    res = run_bass_kernel_spmd(nc, _in_maps(inputs), core_ids=list(range(8)))
    return res


def measure_hw_ns(inputs, M=64, reps=1):
    import time
    import jax
    from jax.sharding import Mesh, PartitionSpec, NamedSharding
    from jax.experimental.shard_map import shard_map
    from concourse import mybir
    from concourse.bass2jax import _bass_exec_p, install_neuronx_cc_hook, partition_id_tensor

    nc = _get_nc(reps=reps)
    install_neuronx_cc_hook()
    in_maps = _in_maps(inputs)
    partition_name = nc.partition_id_tensor.name if nc.partition_id_tensor else None
    in_names, out_names, out_avals, zero_outs = [], [], [], []
    for alloc in nc.m.functions[0].allocations:
        if not isinstance(alloc, mybir.MemoryLocationSet):
            continue
        name = alloc.memorylocations[0].name
        if alloc.kind == "ExternalInput":
            if name != partition_name:
                in_names.append(name)
        elif alloc.kind == "ExternalOutput":
            out_names.append(name)
            shape = tuple(alloc.tensor_shape)
            dtype = mybir.dt.np(alloc.dtype)
            out_avals.append(jax.core.ShapedArray(shape, dtype))
            zero_outs.append(np.zeros(shape, dtype))
    n_params = len(in_names)
    in_names_all = in_names + out_names
    if partition_name is not None:
        in_names_all.append(partition_name)

    def _body(*args):
        operands = list(args)
        if partition_name is not None:
            operands.append(partition_id_tensor())
        outs = _bass_exec_p.bind(
            *operands, out_avals=tuple(out_avals), in_names=tuple(in_names_all),
            out_names=tuple(out_names), lowering_input_output_aliases=(),
            sim_require_finite=True, sim_require_nnan=True, nc=nc)
        return tuple(outs)

    devices = jax.devices()[:8]
    mesh = Mesh(np.asarray(devices), ("core",))
    spec = PartitionSpec("core")
    fn = jax.jit(
        shard_map(_body, mesh=mesh, in_specs=(spec,) * (n_params + len(out_avals)),
                  out_specs=(spec,) * len(out_avals), check_rep=False),
        keep_unused=True)
    per_core = [[np.asarray(m[name]) for name in in_names] for m in in_maps]
    concat_in = [np.concatenate([per_core[c][i] for c in range(8)], axis=0)
                 for i in range(n_params)]
    concat_zeros = [np.zeros((8 * zz.shape[0], *zz.shape[1:]), zz.dtype) for zz in zero_outs]
    sh = NamedSharding(mesh, spec)
    dev_in = [jax.device_put(a, sh) for a in concat_in]
    dev_zero = [jax.device_put(a, sh) for a in concat_zeros]
    o = fn(*dev_in, *dev_zero)
    jax.block_until_ready(o)
    t0 = time.perf_counter()
    outs = [fn(*dev_in, *dev_zero) for _ in range(M)]
    jax.block_until_ready(outs)
    t1 = time.perf_counter()
    return (t1 - t0) / M * 1e9

